# revision 7
# baseline (speedup 1.0000x reference)
"""ButterflyLinear kernel for 8 TRN2 NeuronCores.

All 12 butterfly stages in the reference use the same adjacent-pair
grouping, so the scan collapses into a single per-pair 2x2 transform
C[n] = F_0[n] @ F_1[n] @ ... @ F_11[n] (times alpha).  The device kernel
composes C from the factors on-chip, then streams x through one
elementwise pass:

    out[:, 2n]   = x[:, 2n] * C[n,0,0] + x[:, 2n+1] * C[n,1,0]
    out[:, 2n+1] = x[:, 2n] * C[n,0,1] + x[:, 2n+1] * C[n,1,1]

Data-parallel over the flattened batch*seq dim: 16384 rows -> 8 cores x
2048 rows.  factors/alpha are replicated.
"""

import sys

if "/opt/trn_rl_repo" not in sys.path:
    sys.path.insert(0, "/opt/trn_rl_repo")

import numpy as np

import concourse.mybir as mybir
from concourse import bacc, bass
from concourse.bass import Bass
from concourse.bass_utils import run_bass_kernel_spmd
from concourse.tile import TileContext

B, S, N = 4, 4096, 4096
M = B * S                  # 16384 flattened rows
NCORES = 8
M_SHARD = M // NCORES      # 2048 rows per core
P = 128                    # partitions
TILES = M_SHARD // P       # 16 row-tiles per core
HALF = N // 2              # 2048 pairs
F = 12                     # butterfly factors
FP32 = mybir.dt.float32


def _build_bass(loop_reps: int = 1) -> Bass:
    """Build the SPMD program.  loop_reps > 1 wraps the streaming pass in a
    hardware For-loop (benchmarking only — output is rewritten each rep)."""
    nc = bacc.Bacc("TRN2", target_bir_lowering=False)

    x = nc.declare_dram_parameter("x", [M_SHARD, N], FP32, isOutput=False)
    factors = nc.declare_dram_parameter("factors", [F, HALF, 2, 2], FP32,
                                        isOutput=False)
    alpha = nc.declare_dram_parameter("alpha", [1], FP32, isOutput=False)
    out = nc.declare_dram_parameter("out", [M_SHARD, N], FP32, isOutput=True)

    with TileContext(nc) as tc:
        from contextlib import ExitStack
        with ExitStack() as ctx:
            singles = ctx.enter_context(tc.tile_pool(name="singles", bufs=1))
            dram = ctx.enter_context(
                tc.tile_pool(name="dram", bufs=1, space="DRAM"))
            xpool = ctx.enter_context(tc.tile_pool(name="xpool", bufs=3))
            opool = ctx.enter_context(tc.tile_pool(name="opool", bufs=3))
            tpool = ctx.enter_context(tc.tile_pool(name="tpool", bufs=2))

            # ---- Phase 0: load factors --------------------------------
            # fac[p, k*64 + j] = factors[k, p*16 + j//4, (j%4)//2, j%2]
            # (per k: partition p holds blocks n in [p*16, p*16+16), each
            # block 4 contiguous values 00,01,10,11)
            fac = singles.tile([P, F * 64], FP32)
            nc.sync.dma_start(
                out=fac[:, :],
                in_=bass.AP(tensor=factors, offset=0,
                            ap=[[64, P], [64 * P, F], [1, 64]]),
            )

            # alpha, broadcast to [128, 1]
            alpha_t = singles.tile([P, 1], FP32)
            nc.gpsimd.dma_start(
                out=alpha_t[:, :],
                in_=bass.AP(tensor=alpha, offset=0, ap=[[0, P], [1, 1]]),
            )

            # ---- Phase 1: compose C = F_0 @ F_1 @ ... @ F_11 ----------
            # C held as 4 [P, 16] tiles (partition-major over n).
            def fview(k, b, c):
                off = k * 64 + b * 2 + c
                return fac[:, off:k * 64 + 64:4]

            c00 = singles.tile([P, 16], FP32)
            c01 = singles.tile([P, 16], FP32)
            c10 = singles.tile([P, 16], FP32)
            c11 = singles.tile([P, 16], FP32)
            t00 = singles.tile([P, 16], FP32)
            t01 = singles.tile([P, 16], FP32)
            t10 = singles.tile([P, 16], FP32)
            t11 = singles.tile([P, 16], FP32)
            tmp = singles.tile([P, 16], FP32)

            nc.vector.tensor_copy(out=c00[:, :], in_=fview(0, 0, 0))
            nc.vector.tensor_copy(out=c01[:, :], in_=fview(0, 0, 1))
            nc.vector.tensor_copy(out=c10[:, :], in_=fview(0, 1, 0))
            nc.vector.tensor_copy(out=c11[:, :], in_=fview(0, 1, 1))

            cur = (c00, c01, c10, c11)
            nxt = (t00, t01, t10, t11)
            for k in range(1, F):
                a00, a01, a10, a11 = cur
                n00, n01, n10, n11 = nxt
                f00, f01 = fview(k, 0, 0), fview(k, 0, 1)
                f10, f11 = fview(k, 1, 0), fview(k, 1, 1)
                # new00 = a00*f00 + a01*f10 ; new01 = a00*f01 + a01*f11
                # new10 = a10*f00 + a11*f10 ; new11 = a10*f01 + a11*f11
                for (dst, u, fu, v, fv) in (
                    (n00, a00, f00, a01, f10),
                    (n01, a00, f01, a01, f11),
                    (n10, a10, f00, a11, f10),
                    (n11, a10, f01, a11, f11),
                ):
                    nc.vector.tensor_mul(out=dst[:, :], in0=u[:, :], in1=fu)
                    nc.vector.tensor_mul(out=tmp[:, :], in0=v[:, :], in1=fv)
                    nc.vector.tensor_add(out=dst[:, :], in0=dst[:, :],
                                         in1=tmp[:, :])
                cur, nxt = nxt, cur

            # fold alpha, packing the 4 coefficients into one [P, 64] tile
            # (single source for the scratch-write DMA below — keeps the
            # broadcast DMA's wait count at 1)
            c_all = singles.tile([P, 64], FP32)
            for q, t in enumerate(cur):
                nc.vector.tensor_scalar_mul(
                    c_all[:, q * 16:(q + 1) * 16], t[:, :], alpha_t[:, 0:1])

            # ---- Phase 2: reorder to n-major in DRAM, broadcast back --
            cdram = dram.tile([4 * HALF], FP32)
            nc.sync.dma_start(
                out=bass.AP(tensor=cdram.tensor, offset=cdram.offset,
                            ap=[[16, P], [HALF, 4], [1, 16]]),
                in_=c_all[:, :],
            )
            cb = singles.tile([P, 4 * HALF], FP32)
            nc.gpsimd.dma_start(
                out=cb[:, :],
                in_=bass.AP(tensor=cdram.tensor, offset=cdram.offset,
                            ap=[[0, P], [1, 4 * HALF]]),
            )
            c00b = cb[:, 0 * HALF:1 * HALF]
            c01b = cb[:, 1 * HALF:2 * HALF]
            c10b = cb[:, 2 * HALF:3 * HALF]
            c11b = cb[:, 3 * HALF:4 * HALF]

            # ---- Phase 3: stream x ------------------------------------
            def stream_pass(_iv=None):
                for i in range(TILES):
                    xt = xpool.tile([P, N], FP32)
                    nc.sync.dma_start(out=xt[:, :],
                                      in_=x[i * P:(i + 1) * P, :])
                    ot = opool.tile([P, N], FP32)
                    xe = xt[:, 0:N:2]
                    xo = xt[:, 1:N:2]
                    t1 = tpool.tile([P, HALF], FP32)
                    t2 = tpool.tile([P, HALF], FP32)
                    nc.vector.tensor_mul(out=t1[:, :], in0=xe, in1=c00b)
                    nc.vector.tensor_mul(out=t2[:, :], in0=xo, in1=c10b)
                    nc.vector.tensor_add(out=ot[:, 0:N:2], in0=t1[:, :],
                                         in1=t2[:, :])
                    t3 = tpool.tile([P, HALF], FP32)
                    t4 = tpool.tile([P, HALF], FP32)
                    nc.vector.tensor_mul(out=t3[:, :], in0=xe, in1=c01b)
                    nc.vector.tensor_mul(out=t4[:, :], in0=xo, in1=c11b)
                    nc.vector.tensor_add(out=ot[:, 1:N:2], in0=t3[:, :],
                                         in1=t4[:, :])
                    nc.sync.dma_start(out=out[i * P:(i + 1) * P, :],
                                      in_=ot[:, :])

            if loop_reps == 1:
                stream_pass()
            else:
                with tc.For_i(0, loop_reps, 1):
                    stream_pass()

    nc.compile()
    return nc


_CACHE: dict = {}


def _get_nc() -> Bass:
    if "nc" not in _CACHE:
        _CACHE["nc"] = _build_bass()
    return _CACHE["nc"]


def kernel(x: np.ndarray, factors: np.ndarray, alpha: np.ndarray,
           **_kwargs) -> np.ndarray:
    nc = _get_nc()
    x_flat = np.ascontiguousarray(x, dtype=np.float32).reshape(M, N)
    factors = np.ascontiguousarray(factors, dtype=np.float32)
    alpha = np.ascontiguousarray(alpha, dtype=np.float32)

    in_maps = []
    for i in range(NCORES):
        shard = np.ascontiguousarray(x_flat[i * M_SHARD:(i + 1) * M_SHARD])
        in_maps.append({"x": shard, "factors": factors, "alpha": alpha})

    res = run_bass_kernel_spmd(nc, in_maps, core_ids=list(range(NCORES)))
    out = np.concatenate([res.results[i]["out"] for i in range(NCORES)],
                         axis=0)
    return out.reshape(B, S, N)


# revision 10
# speedup vs baseline: 1.4669x; 1.4669x over previous
"""ButterflyLinear kernel for 8 TRN2 NeuronCores.

All 12 butterfly stages in the reference use the same adjacent-pair
grouping, so the scan collapses into a single per-pair 2x2 transform
C[n] = F_0[n] @ F_1[n] @ ... @ F_11[n] (times alpha).  The device kernel
composes C from the factors on-chip, then streams x through one
elementwise pass:

    out[:, 2n]   = x[:, 2n] * C[n,0,0] + x[:, 2n+1] * C[n,1,0]
    out[:, 2n+1] = x[:, 2n] * C[n,0,1] + x[:, 2n+1] * C[n,1,1]

Data-parallel over the flattened batch*seq dim: 16384 rows -> 8 cores x
2048 rows.  factors/alpha are replicated.
"""

import sys

if "/opt/trn_rl_repo" not in sys.path:
    sys.path.insert(0, "/opt/trn_rl_repo")

import numpy as np

import concourse.mybir as mybir
from concourse import bacc, bass
from concourse.bass import Bass
from concourse.bass_utils import run_bass_kernel_spmd
from concourse.tile import TileContext

B, S, N = 4, 4096, 4096
M = B * S                  # 16384 flattened rows
NCORES = 8
M_SHARD = M // NCORES      # 2048 rows per core
P = 128                    # partitions
TILES = M_SHARD // P       # 16 row-tiles per core
HALF = N // 2              # 2048 pairs
F = 12                     # butterfly factors
FP32 = mybir.dt.float32


def _build_bass(loop_reps: int = 1, variant: str = "full") -> Bass:
    """Build the SPMD program.  loop_reps > 1 wraps the streaming pass in a
    hardware For-loop (benchmarking only — output is rewritten each rep).
    variant: "full" | "dma" (no compute) | "dve" (no x load / out store)."""
    nc = bacc.Bacc("TRN2", target_bir_lowering=False)

    x = nc.declare_dram_parameter("x", [M_SHARD, N], FP32, isOutput=False)
    factors = nc.declare_dram_parameter("factors", [F, HALF, 2, 2], FP32,
                                        isOutput=False)
    alpha = nc.declare_dram_parameter("alpha", [1], FP32, isOutput=False)
    out = nc.declare_dram_parameter("out", [M_SHARD, N], FP32, isOutput=True)

    with TileContext(nc) as tc:
        from contextlib import ExitStack
        with ExitStack() as ctx:
            singles = ctx.enter_context(tc.tile_pool(name="singles", bufs=1))
            dram = ctx.enter_context(
                tc.tile_pool(name="dram", bufs=1, space="DRAM"))
            xpool = ctx.enter_context(tc.tile_pool(name="xpool", bufs=3))
            opool = ctx.enter_context(tc.tile_pool(name="opool", bufs=3))
            tpool = ctx.enter_context(tc.tile_pool(name="tpool", bufs=2))

            # ---- Phase 0: load factors --------------------------------
            # fac[p, k*64 + j] = factors[k, p*16 + j//4, (j%4)//2, j%2]
            # (per k: partition p holds blocks n in [p*16, p*16+16), each
            # block 4 contiguous values 00,01,10,11)
            fac = singles.tile([P, F * 64], FP32)
            nc.sync.dma_start(
                out=fac[:, :],
                in_=bass.AP(tensor=factors, offset=0,
                            ap=[[64, P], [64 * P, F], [1, 64]]),
            )

            # alpha, broadcast to [128, 1]
            alpha_t = singles.tile([P, 1], FP32)
            nc.gpsimd.dma_start(
                out=alpha_t[:, :],
                in_=bass.AP(tensor=alpha, offset=0, ap=[[0, P], [1, 1]]),
            )

            # ---- Phase 1: compose C = F_0 @ F_1 @ ... @ F_11 ----------
            # C held as 4 [P, 16] tiles (partition-major over n).
            def fview(k, b, c):
                off = k * 64 + b * 2 + c
                return fac[:, off:k * 64 + 64:4]

            c00 = singles.tile([P, 16], FP32)
            c01 = singles.tile([P, 16], FP32)
            c10 = singles.tile([P, 16], FP32)
            c11 = singles.tile([P, 16], FP32)
            t00 = singles.tile([P, 16], FP32)
            t01 = singles.tile([P, 16], FP32)
            t10 = singles.tile([P, 16], FP32)
            t11 = singles.tile([P, 16], FP32)
            tmp = singles.tile([P, 16], FP32)

            nc.vector.tensor_copy(out=c00[:, :], in_=fview(0, 0, 0))
            nc.vector.tensor_copy(out=c01[:, :], in_=fview(0, 0, 1))
            nc.vector.tensor_copy(out=c10[:, :], in_=fview(0, 1, 0))
            nc.vector.tensor_copy(out=c11[:, :], in_=fview(0, 1, 1))

            cur = (c00, c01, c10, c11)
            nxt = (t00, t01, t10, t11)
            for k in range(1, F):
                a00, a01, a10, a11 = cur
                n00, n01, n10, n11 = nxt
                f00, f01 = fview(k, 0, 0), fview(k, 0, 1)
                f10, f11 = fview(k, 1, 0), fview(k, 1, 1)
                # new00 = a00*f00 + a01*f10 ; new01 = a00*f01 + a01*f11
                # new10 = a10*f00 + a11*f10 ; new11 = a10*f01 + a11*f11
                for (dst, u, fu, v, fv) in (
                    (n00, a00, f00, a01, f10),
                    (n01, a00, f01, a01, f11),
                    (n10, a10, f00, a11, f10),
                    (n11, a10, f01, a11, f11),
                ):
                    nc.vector.tensor_mul(out=dst[:, :], in0=u[:, :], in1=fu)
                    nc.vector.tensor_mul(out=tmp[:, :], in0=v[:, :], in1=fv)
                    nc.vector.tensor_add(out=dst[:, :], in0=dst[:, :],
                                         in1=tmp[:, :])
                cur, nxt = nxt, cur

            # fold alpha, packing the 4 coefficients into one [P, 64] tile
            # (single source for the scratch-write DMA below — keeps the
            # broadcast DMA's wait count at 1)
            c_all = singles.tile([P, 64], FP32)
            for q, t in enumerate(cur):
                nc.vector.tensor_scalar_mul(
                    c_all[:, q * 16:(q + 1) * 16], t[:, :], alpha_t[:, 0:1])

            # ---- Phase 2: reorder to n-major in DRAM, broadcast back --
            cdram = dram.tile([4 * HALF], FP32)
            nc.sync.dma_start(
                out=bass.AP(tensor=cdram.tensor, offset=cdram.offset,
                            ap=[[16, P], [HALF, 4], [1, 16]]),
                in_=c_all[:, :],
            )
            cb = singles.tile([P, 4 * HALF], FP32)
            nc.gpsimd.dma_start(
                out=cb[:, :],
                in_=bass.AP(tensor=cdram.tensor, offset=cdram.offset,
                            ap=[[0, P], [1, 4 * HALF]]),
            )
            c00b = cb[:, 0 * HALF:1 * HALF]
            c01b = cb[:, 1 * HALF:2 * HALF]
            c10b = cb[:, 2 * HALF:3 * HALF]
            c11b = cb[:, 3 * HALF:4 * HALF]

            # ---- Phase 3: stream x ------------------------------------
            def stream_pass(_iv=None):
                for i in range(TILES):
                    xt = xpool.tile([P, N], FP32)
                    if variant != "dve":
                        nc.sync.dma_start(out=xt[:, :],
                                          in_=x[i * P:(i + 1) * P, :])
                    ot = opool.tile([P, N], FP32)
                    if variant != "dma":
                        xe = xt[:, 0:N:2]
                        xo = xt[:, 1:N:2]
                        eng2 = (nc.gpsimd if variant in ("gps", "split")
                                else nc.vector)
                        eng1 = nc.gpsimd if variant == "gps" else nc.vector
                        t1 = tpool.tile([P, HALF], FP32)
                        t2 = tpool.tile([P, HALF], FP32)
                        eng1.tensor_mul(out=t1[:, :], in0=xe, in1=c00b)
                        eng1.tensor_mul(out=t2[:, :], in0=xo, in1=c10b)
                        eng1.tensor_add(out=ot[:, 0:N:2], in0=t1[:, :],
                                        in1=t2[:, :])
                        t3 = tpool.tile([P, HALF], FP32)
                        t4 = tpool.tile([P, HALF], FP32)
                        eng2.tensor_mul(out=t3[:, :], in0=xe, in1=c01b)
                        eng2.tensor_mul(out=t4[:, :], in0=xo, in1=c11b)
                        eng1.tensor_add(out=ot[:, 1:N:2], in0=t3[:, :],
                                        in1=t4[:, :])
                    if variant != "dve":
                        nc.sync.dma_start(out=out[i * P:(i + 1) * P, :],
                                          in_=ot[:, :])

            if loop_reps == 1:
                stream_pass()
            else:
                with tc.For_i(0, loop_reps, 1):
                    stream_pass()

    nc.compile()
    return nc


_CACHE: dict = {}


def _get_nc() -> Bass:
    if "nc" not in _CACHE:
        _CACHE["nc"] = _build_bass()
    return _CACHE["nc"]


def kernel(x: np.ndarray, factors: np.ndarray, alpha: np.ndarray,
           **_kwargs) -> np.ndarray:
    nc = _get_nc()
    x_flat = np.ascontiguousarray(x, dtype=np.float32).reshape(M, N)
    factors = np.ascontiguousarray(factors, dtype=np.float32)
    alpha = np.ascontiguousarray(alpha, dtype=np.float32)

    in_maps = []
    for i in range(NCORES):
        shard = np.ascontiguousarray(x_flat[i * M_SHARD:(i + 1) * M_SHARD])
        in_maps.append({"x": shard, "factors": factors, "alpha": alpha})

    res = run_bass_kernel_spmd(nc, in_maps, core_ids=list(range(NCORES)))
    out = np.concatenate([res.results[i]["out"] for i in range(NCORES)],
                         axis=0)
    return out.reshape(B, S, N)


# revision 15
# speedup vs baseline: 1.5296x; 1.0428x over previous
"""ButterflyLinear kernel for 8 TRN2 NeuronCores.

All 12 butterfly stages in the reference use the same adjacent-pair
grouping, so the scan collapses into a single per-pair 2x2 transform
C[n] = F_0[n] @ F_1[n] @ ... @ F_11[n] (times alpha).  The device kernel
composes C from the factors on-chip, then streams x through one
elementwise pass:

    out[:, 2n]   = x[:, 2n] * C[n,0,0] + x[:, 2n+1] * C[n,1,0]
    out[:, 2n+1] = x[:, 2n] * C[n,0,1] + x[:, 2n+1] * C[n,1,1]

Data-parallel over the flattened batch*seq dim: 16384 rows -> 8 cores x
2048 rows.  factors/alpha are replicated.
"""

import sys

if "/opt/trn_rl_repo" not in sys.path:
    sys.path.insert(0, "/opt/trn_rl_repo")

import numpy as np

import concourse.mybir as mybir
from concourse import bacc, bass
from concourse.bass import Bass
from concourse.bass_utils import run_bass_kernel_spmd
from concourse.tile import TileContext

B, S, N = 4, 4096, 4096
M = B * S                  # 16384 flattened rows
NCORES = 8
M_SHARD = M // NCORES      # 2048 rows per core
P = 128                    # partitions
TILES = M_SHARD // P       # 16 row-tiles per core
HALF = N // 2              # 2048 pairs
F = 12                     # butterfly factors
FP32 = mybir.dt.float32


def _build_bass(loop_reps: int = 1, variant: str = "full",
                loop_scope: str = "pass") -> Bass:
    """Build the SPMD program.  loop_reps > 1 wraps the streaming pass in a
    hardware For-loop (benchmarking only — output is rewritten each rep).
    variant: "full" | "dma" (no compute) | "dve" (no x load / out store)
             | "gps" (all elementwise on GpSimd) | "split" (DVE+GpSimd).
    loop_scope: "pass" loops only the streaming pass; "all" also re-runs
    the coefficient setup every rep."""
    nc = bacc.Bacc("TRN2", target_bir_lowering=False)

    x = nc.declare_dram_parameter("x", [M_SHARD, N], FP32, isOutput=False)
    factors = nc.declare_dram_parameter("factors", [F, HALF, 2, 2], FP32,
                                        isOutput=False)
    alpha = nc.declare_dram_parameter("alpha", [1], FP32, isOutput=False)
    out = nc.declare_dram_parameter("out", [M_SHARD, N], FP32, isOutput=True)

    with TileContext(nc) as tc:
        from contextlib import ExitStack
        with ExitStack() as ctx:
            singles = ctx.enter_context(tc.tile_pool(name="singles", bufs=1))
            dram = ctx.enter_context(
                tc.tile_pool(name="dram", bufs=1, space="DRAM"))
            xpool = ctx.enter_context(tc.tile_pool(name="xpool", bufs=3))
            opool = ctx.enter_context(tc.tile_pool(name="opool", bufs=3))
            tpool = ctx.enter_context(tc.tile_pool(name="tpool", bufs=2))

            coeffs = {}

            def setup_phase():
                # ---- Phase 0: load factors ----------------------------
                # fac[p, k*64 + j] = factors[k, p*16 + j//4, (j%4)//2, j%2]
                # (per k: partition p holds blocks n in [p*16, p*16+16),
                # each block 4 contiguous values 00,01,10,11)
                fac = singles.tile([P, F * 64], FP32)
                nc.sync.dma_start(
                    out=fac[:, :],
                    in_=bass.AP(tensor=factors, offset=0,
                                ap=[[64, P], [64 * P, F], [1, 64]]),
                )

                # alpha, broadcast to [128, 1]
                alpha_t = singles.tile([P, 1], FP32)
                nc.gpsimd.dma_start(
                    out=alpha_t[:, :],
                    in_=bass.AP(tensor=alpha, offset=0, ap=[[0, P], [1, 1]]),
                )

                # ---- Phase 1: compose C = F_0 @ F_1 @ ... @ F_11 ------
                # C held as 4 [P, 16] tiles (partition-major over n).
                def fview(k, b, c):
                    off = k * 64 + b * 2 + c
                    return fac[:, off:k * 64 + 64:4]

                c00 = singles.tile([P, 16], FP32)
                c01 = singles.tile([P, 16], FP32)
                c10 = singles.tile([P, 16], FP32)
                c11 = singles.tile([P, 16], FP32)
                t00 = singles.tile([P, 16], FP32)
                t01 = singles.tile([P, 16], FP32)
                t10 = singles.tile([P, 16], FP32)
                t11 = singles.tile([P, 16], FP32)
                tmp = singles.tile([P, 16], FP32)

                nc.vector.tensor_copy(out=c00[:, :], in_=fview(0, 0, 0))
                nc.vector.tensor_copy(out=c01[:, :], in_=fview(0, 0, 1))
                nc.vector.tensor_copy(out=c10[:, :], in_=fview(0, 1, 0))
                nc.vector.tensor_copy(out=c11[:, :], in_=fview(0, 1, 1))

                cur = (c00, c01, c10, c11)
                nxt = (t00, t01, t10, t11)
                for k in range(1, F):
                    a00, a01, a10, a11 = cur
                    n00, n01, n10, n11 = nxt
                    f00, f01 = fview(k, 0, 0), fview(k, 0, 1)
                    f10, f11 = fview(k, 1, 0), fview(k, 1, 1)
                    # new00 = a00*f00 + a01*f10 ; new01 = a00*f01 + a01*f11
                    # new10 = a10*f00 + a11*f10 ; new11 = a10*f01 + a11*f11
                    for (dst, u, fu, v, fv) in (
                        (n00, a00, f00, a01, f10),
                        (n01, a00, f01, a01, f11),
                        (n10, a10, f00, a11, f10),
                        (n11, a10, f01, a11, f11),
                    ):
                        nc.vector.tensor_mul(out=dst[:, :], in0=u[:, :],
                                             in1=fu)
                        nc.vector.tensor_mul(out=tmp[:, :], in0=v[:, :],
                                             in1=fv)
                        nc.vector.tensor_add(out=dst[:, :], in0=dst[:, :],
                                             in1=tmp[:, :])
                    cur, nxt = nxt, cur

                # fold alpha, packing the 4 coefficients into one [P, 64]
                # tile (single source for the scratch-write DMA below —
                # keeps the broadcast DMA's wait count at 1)
                c_all = singles.tile([P, 64], FP32)
                for q, t in enumerate(cur):
                    nc.vector.tensor_scalar_mul(
                        c_all[:, q * 16:(q + 1) * 16], t[:, :],
                        alpha_t[:, 0:1])

                # ---- Phase 2: reorder to n-major in DRAM, broadcast ---
                cdram = dram.tile([4 * HALF], FP32)
                nc.sync.dma_start(
                    out=bass.AP(tensor=cdram.tensor, offset=cdram.offset,
                                ap=[[16, P], [HALF, 4], [1, 16]]),
                    in_=c_all[:, :],
                )
                cb = singles.tile([P, 4 * HALF], FP32)
                nc.gpsimd.dma_start(
                    out=cb[:, :],
                    in_=bass.AP(tensor=cdram.tensor, offset=cdram.offset,
                                ap=[[0, P], [1, 4 * HALF]]),
                )
                coeffs["c00b"] = cb[:, 0 * HALF:1 * HALF]
                coeffs["c01b"] = cb[:, 1 * HALF:2 * HALF]
                coeffs["c10b"] = cb[:, 2 * HALF:3 * HALF]
                coeffs["c11b"] = cb[:, 3 * HALF:4 * HALF]

            # ---- Phase 3: stream x ------------------------------------
            if variant == "dve":
                xt_fixed = singles.tile([P, N], FP32)
                nc.vector.memset(xt_fixed[:, :], 0.5)

            def stream_pass(_iv=None):
                for i in range(TILES):
                    if variant == "dve":
                        xt = xt_fixed
                    else:
                        xt = xpool.tile([P, N], FP32)
                        nc.sync.dma_start(out=xt[:, :],
                                          in_=x[i * P:(i + 1) * P, :])
                    if variant == "dma":
                        nc.sync.dma_start(out=out[i * P:(i + 1) * P, :],
                                          in_=xt[:, :])
                        continue
                    ot = opool.tile([P, N], FP32)
                    xe = xt[:, 0:N:2]
                    xo = xt[:, 1:N:2]
                    c00b, c01b = coeffs["c00b"], coeffs["c01b"]
                    c10b, c11b = coeffs["c10b"], coeffs["c11b"]
                    eng2 = (nc.gpsimd if variant in ("gps", "split")
                            else nc.vector)
                    eng1 = nc.gpsimd if variant == "gps" else nc.vector
                    t1 = tpool.tile([P, HALF], FP32)
                    t2 = tpool.tile([P, HALF], FP32)
                    eng1.tensor_mul(out=t1[:, :], in0=xe, in1=c00b)
                    eng1.tensor_mul(out=t2[:, :], in0=xo, in1=c10b)
                    eng1.tensor_add(out=ot[:, 0:N:2], in0=t1[:, :],
                                    in1=t2[:, :])
                    t3 = tpool.tile([P, HALF], FP32)
                    t4 = tpool.tile([P, HALF], FP32)
                    eng2.tensor_mul(out=t3[:, :], in0=xe, in1=c01b)
                    eng2.tensor_mul(out=t4[:, :], in0=xo, in1=c11b)
                    eng1.tensor_add(out=ot[:, 1:N:2], in0=t3[:, :],
                                    in1=t4[:, :])
                    if variant != "dve":
                        nc.sync.dma_start(out=out[i * P:(i + 1) * P, :],
                                          in_=ot[:, :])

            if loop_scope == "all" and loop_reps > 1:
                with tc.For_i(0, loop_reps, 1):
                    setup_phase()
                    stream_pass()
            else:
                setup_phase()
                if loop_reps == 1:
                    stream_pass()
                else:
                    with tc.For_i(0, loop_reps, 1):
                        stream_pass()

    nc.compile()
    return nc


_CACHE: dict = {}


def _get_nc() -> Bass:
    if "nc" not in _CACHE:
        _CACHE["nc"] = _build_bass()
    return _CACHE["nc"]


def kernel(x: np.ndarray, factors: np.ndarray, alpha: np.ndarray,
           **_kwargs) -> np.ndarray:
    nc = _get_nc()
    x_flat = np.ascontiguousarray(x, dtype=np.float32).reshape(M, N)
    factors = np.ascontiguousarray(factors, dtype=np.float32)
    alpha = np.ascontiguousarray(alpha, dtype=np.float32)

    in_maps = []
    for i in range(NCORES):
        shard = np.ascontiguousarray(x_flat[i * M_SHARD:(i + 1) * M_SHARD])
        in_maps.append({"x": shard, "factors": factors, "alpha": alpha})

    res = run_bass_kernel_spmd(nc, in_maps, core_ids=list(range(NCORES)))
    out = np.concatenate([res.results[i]["out"] for i in range(NCORES)],
                         axis=0)
    return out.reshape(B, S, N)


# revision 16
# speedup vs baseline: 1.7992x; 1.1762x over previous
"""ButterflyLinear kernel for 8 TRN2 NeuronCores.

All 12 butterfly stages in the reference use the same adjacent-pair
grouping, so the scan collapses into a single per-pair 2x2 transform
C[n] = F_0[n] @ F_1[n] @ ... @ F_11[n] (times alpha).  The device kernel
composes C from the factors on-chip, then streams x through one
elementwise pass:

    out[:, 2n]   = x[:, 2n] * C[n,0,0] + x[:, 2n+1] * C[n,1,0]
    out[:, 2n+1] = x[:, 2n] * C[n,0,1] + x[:, 2n+1] * C[n,1,1]

Data-parallel over the flattened batch*seq dim: 16384 rows -> 8 cores x
2048 rows.  factors/alpha are replicated.
"""

import sys

if "/opt/trn_rl_repo" not in sys.path:
    sys.path.insert(0, "/opt/trn_rl_repo")

import numpy as np

import concourse.mybir as mybir
from concourse import bacc, bass
from concourse.bass import Bass
from concourse.bass_utils import run_bass_kernel_spmd
from concourse.tile import TileContext

B, S, N = 4, 4096, 4096
M = B * S                  # 16384 flattened rows
NCORES = 8
M_SHARD = M // NCORES      # 2048 rows per core
P = 128                    # partitions
TILES = M_SHARD // P       # 16 row-tiles per core
HALF = N // 2              # 2048 pairs
F = 12                     # butterfly factors
FP32 = mybir.dt.float32


def _build_bass(loop_reps: int = 1, variant: str = "full",
                loop_scope: str = "pass") -> Bass:
    """Build the SPMD program.  loop_reps > 1 wraps the streaming pass in a
    hardware For-loop (benchmarking only — output is rewritten each rep).
    variant: "full" | "dma" (no compute) | "dve" (no x load / out store)
             | "gps" (all elementwise on GpSimd) | "split" (DVE+GpSimd).
    loop_scope: "pass" loops only the streaming pass; "all" also re-runs
    the coefficient setup every rep."""
    nc = bacc.Bacc("TRN2", target_bir_lowering=False)

    x = nc.declare_dram_parameter("x", [M_SHARD, N], FP32, isOutput=False)
    factors = nc.declare_dram_parameter("factors", [F, HALF, 2, 2], FP32,
                                        isOutput=False)
    alpha = nc.declare_dram_parameter("alpha", [1], FP32, isOutput=False)
    out = nc.declare_dram_parameter("out", [M_SHARD, N], FP32, isOutput=True)

    with TileContext(nc) as tc:
        from contextlib import ExitStack
        with ExitStack() as ctx:
            singles = ctx.enter_context(tc.tile_pool(name="singles", bufs=1))
            dram = ctx.enter_context(
                tc.tile_pool(name="dram", bufs=1, space="DRAM"))
            xpool = ctx.enter_context(tc.tile_pool(name="xpool", bufs=3))
            opool = ctx.enter_context(tc.tile_pool(name="opool", bufs=3))
            tpool = ctx.enter_context(tc.tile_pool(name="tpool", bufs=2))

            coeffs = {}

            def setup_phase():
                # ---- Phase 0: load factors ----------------------------
                # fac[p, k*64 + j] = factors[k, p*16 + j//4, (j%4)//2, j%2]
                # (per k: partition p holds blocks n in [p*16, p*16+16),
                # each block 4 contiguous values 00,01,10,11)
                fac = singles.tile([P, F * 64], FP32)
                nc.sync.dma_start(
                    out=fac[:, :],
                    in_=bass.AP(tensor=factors, offset=0,
                                ap=[[64, P], [64 * P, F], [1, 64]]),
                )

                # alpha, broadcast to [128, 1]
                alpha_t = singles.tile([P, 1], FP32)
                nc.gpsimd.dma_start(
                    out=alpha_t[:, :],
                    in_=bass.AP(tensor=alpha, offset=0, ap=[[0, P], [1, 1]]),
                )

                # ---- Phase 1: compose C = F_0 @ F_1 @ ... @ F_11 ------
                # C held as 4 [P, 16] tiles (partition-major over n).
                def fview(k, b, c):
                    off = k * 64 + b * 2 + c
                    return fac[:, off:k * 64 + 64:4]

                c00 = singles.tile([P, 16], FP32)
                c01 = singles.tile([P, 16], FP32)
                c10 = singles.tile([P, 16], FP32)
                c11 = singles.tile([P, 16], FP32)
                t00 = singles.tile([P, 16], FP32)
                t01 = singles.tile([P, 16], FP32)
                t10 = singles.tile([P, 16], FP32)
                t11 = singles.tile([P, 16], FP32)
                tmp = singles.tile([P, 16], FP32)

                nc.vector.tensor_copy(out=c00[:, :], in_=fview(0, 0, 0))
                nc.vector.tensor_copy(out=c01[:, :], in_=fview(0, 0, 1))
                nc.vector.tensor_copy(out=c10[:, :], in_=fview(0, 1, 0))
                nc.vector.tensor_copy(out=c11[:, :], in_=fview(0, 1, 1))

                cur = (c00, c01, c10, c11)
                nxt = (t00, t01, t10, t11)
                for k in range(1, F):
                    a00, a01, a10, a11 = cur
                    n00, n01, n10, n11 = nxt
                    f00, f01 = fview(k, 0, 0), fview(k, 0, 1)
                    f10, f11 = fview(k, 1, 0), fview(k, 1, 1)
                    # new00 = a00*f00 + a01*f10 ; new01 = a00*f01 + a01*f11
                    # new10 = a10*f00 + a11*f10 ; new11 = a10*f01 + a11*f11
                    for (dst, u, fu, v, fv) in (
                        (n00, a00, f00, a01, f10),
                        (n01, a00, f01, a01, f11),
                        (n10, a10, f00, a11, f10),
                        (n11, a10, f01, a11, f11),
                    ):
                        nc.vector.tensor_mul(out=dst[:, :], in0=u[:, :],
                                             in1=fu)
                        nc.vector.tensor_mul(out=tmp[:, :], in0=v[:, :],
                                             in1=fv)
                        nc.vector.tensor_add(out=dst[:, :], in0=dst[:, :],
                                             in1=tmp[:, :])
                    cur, nxt = nxt, cur

                # fold alpha, packing the 4 coefficients into one [P, 64]
                # tile (single source for the scratch-write DMA below —
                # keeps the broadcast DMA's wait count at 1)
                c_all = singles.tile([P, 64], FP32)
                for q, t in enumerate(cur):
                    nc.vector.tensor_scalar_mul(
                        c_all[:, q * 16:(q + 1) * 16], t[:, :],
                        alpha_t[:, 0:1])

                # ---- Phase 2: reorder to n-major in DRAM, broadcast ---
                cdram = dram.tile([4 * HALF], FP32)
                nc.sync.dma_start(
                    out=bass.AP(tensor=cdram.tensor, offset=cdram.offset,
                                ap=[[16, P], [HALF, 4], [1, 16]]),
                    in_=c_all[:, :],
                )
                cb = singles.tile([P, 4 * HALF], FP32)
                nc.gpsimd.dma_start(
                    out=cb[:, :],
                    in_=bass.AP(tensor=cdram.tensor, offset=cdram.offset,
                                ap=[[0, P], [1, 4 * HALF]]),
                )
                coeffs["c00b"] = cb[:, 0 * HALF:1 * HALF]
                coeffs["c01b"] = cb[:, 1 * HALF:2 * HALF]
                coeffs["c10b"] = cb[:, 2 * HALF:3 * HALF]
                coeffs["c11b"] = cb[:, 3 * HALF:4 * HALF]

            # ---- Phase 3: stream x ------------------------------------
            if variant == "dve":
                xt_fixed = singles.tile([P, N], FP32)
                nc.vector.memset(xt_fixed[:, :], 0.5)

            def stream_pass(_iv=None):
                for i in range(TILES):
                    if variant == "dve":
                        xt = xt_fixed
                    else:
                        xt = xpool.tile([P, N], FP32)
                        nc.sync.dma_start(out=xt[:, :],
                                          in_=x[i * P:(i + 1) * P, :])
                    if variant == "dma":
                        nc.scalar.dma_start(out=out[i * P:(i + 1) * P, :],
                                            in_=xt[:, :])
                        continue
                    ot = opool.tile([P, N], FP32)
                    xe = xt[:, 0:N:2]
                    xo = xt[:, 1:N:2]
                    c00b, c01b = coeffs["c00b"], coeffs["c01b"]
                    c10b, c11b = coeffs["c10b"], coeffs["c11b"]
                    eng2 = (nc.gpsimd if variant in ("gps", "split")
                            else nc.vector)
                    eng1 = nc.gpsimd if variant == "gps" else nc.vector
                    t1 = tpool.tile([P, HALF], FP32)
                    t2 = tpool.tile([P, HALF], FP32)
                    eng1.tensor_mul(out=t1[:, :], in0=xe, in1=c00b)
                    eng1.tensor_mul(out=t2[:, :], in0=xo, in1=c10b)
                    eng1.tensor_add(out=ot[:, 0:N:2], in0=t1[:, :],
                                    in1=t2[:, :])
                    t3 = tpool.tile([P, HALF], FP32)
                    t4 = tpool.tile([P, HALF], FP32)
                    eng2.tensor_mul(out=t3[:, :], in0=xe, in1=c01b)
                    eng2.tensor_mul(out=t4[:, :], in0=xo, in1=c11b)
                    eng1.tensor_add(out=ot[:, 1:N:2], in0=t3[:, :],
                                    in1=t4[:, :])
                    if variant != "dve":
                        nc.scalar.dma_start(out=out[i * P:(i + 1) * P, :],
                                            in_=ot[:, :])

            if loop_scope == "all" and loop_reps > 1:
                with tc.For_i(0, loop_reps, 1):
                    setup_phase()
                    stream_pass()
            else:
                setup_phase()
                if loop_reps == 1:
                    stream_pass()
                else:
                    with tc.For_i(0, loop_reps, 1):
                        stream_pass()

    nc.compile()
    return nc


_CACHE: dict = {}


def _get_nc() -> Bass:
    if "nc" not in _CACHE:
        _CACHE["nc"] = _build_bass()
    return _CACHE["nc"]


def kernel(x: np.ndarray, factors: np.ndarray, alpha: np.ndarray,
           **_kwargs) -> np.ndarray:
    nc = _get_nc()
    x_flat = np.ascontiguousarray(x, dtype=np.float32).reshape(M, N)
    factors = np.ascontiguousarray(factors, dtype=np.float32)
    alpha = np.ascontiguousarray(alpha, dtype=np.float32)

    in_maps = []
    for i in range(NCORES):
        shard = np.ascontiguousarray(x_flat[i * M_SHARD:(i + 1) * M_SHARD])
        in_maps.append({"x": shard, "factors": factors, "alpha": alpha})

    res = run_bass_kernel_spmd(nc, in_maps, core_ids=list(range(NCORES)))
    out = np.concatenate([res.results[i]["out"] for i in range(NCORES)],
                         axis=0)
    return out.reshape(B, S, N)
